# revision 2
# baseline (speedup 1.0000x reference)
"""2-layer GCN on 8 TRN2 NeuronCores — SBUF-resident-table gather design.

v2 strategy (vs baseline kernel.py): HBM dma_gather of 256B rows runs at only
~45 GB/s/core (random-read latency bound), while SBUF-source dma_gather runs
at ~450 GB/s/core. So:
  - The full x table (bf16, wrap layout: node v at partition v%128, rank
    v//128, 256B/row) is streamed contiguously into SBUF once (~36us), and
    per-edge rows are gathered SBUF->SBUF with transpose=True.
  - Transposed gather output is [f, e]; each 128-edge chunk is transposed
    back to [e, f] by the PE (identity matmul into PSUM, 8-chunk groups) and
    copied PSUM->SBUF by the Act engine.
  - S one-hots are built per chunk on the DVE: S[e,d] = (iota[d] == md[e,c]),
    one tensor_scalar(is_equal) per chunk. Pad edges have md=-1 (S col = 0)
    and gather row 0 (finite garbage, zero-weighted).
  - agg[f,d] = sum_e msg[e,f] S[e,d] per chunk on the PE as before; dense
    W1/relu/W2 per slot unchanged from baseline.
  - t2 shards AllGather through DRAM per ccsplit region, then each region is
    DMA'd into the same SBUF table tile in wrap layout (rank = core*nblk +
    slot), overwriting x after layer-1 gathers complete (Tile WAR dep).
"""

import os
import sys

import numpy as np

for _p in ("/opt/trn_rl_repo", "/root/.axon_site/_ro/trn_rl_repo"):
    if os.path.isdir(_p) and _p not in sys.path:
        sys.path.append(_p)

import ml_dtypes  # noqa: E402
import concourse.bacc as bacc  # noqa: E402
import concourse.mybir as mybir  # noqa: E402
from concourse.bass_utils import run_bass_kernel_spmd  # noqa: E402
from concourse.tile import TileContext  # noqa: E402
from contextlib import ExitStack  # noqa: E402

BF16 = ml_dtypes.bfloat16
F32 = np.float32
NCORE = 8
GROUP = 6   # dst-block slots per gather batch
GCAP = 14   # max chunks (128 descriptors each) per dma_gather call
TG = 8      # chunks per transpose/copy/S group (one PSUM bank)
SWDGE_SCRATCH = 65536  # 4096-desc ring (>= 2 concurrent GCAP=14 calls)
NQUEUES = 1  # concurrent SBUF-source gathers on distinct queues corrupt each other on HW
LO_SPLIT = 32768  # int16 gather-index limit -> lo/hi table split
ALU = mybir.AluOpType
AF = mybir.ActivationFunctionType

TRACE = False
LAST_EXEC_NS = None
LAST_RESULTS = None


# --------------------------------------------------------------------------
# schedule (identical to baseline)
# --------------------------------------------------------------------------

def _make_schedule(LO, HI, nblk, group):
    batches = []
    c = 0
    for s0 in range(0, nblk, group):
        slots = list(range(s0, min(s0 + group, nblk)))
        lo_entries = []
        lo_base, col = c, 0
        for j in slots:
            lo_entries.append((j, col, c, int(LO[j])))
            col += int(LO[j])
            c += int(LO[j])
        n_lo = col
        hi_entries = []
        hi_base, col = c, 0
        for j in slots:
            hi_entries.append((j, col, c, int(HI[j])))
            col += int(HI[j])
            c += int(HI[j])
        n_hi = col
        batches.append(dict(slots=slots, lo=lo_entries, hi=hi_entries,
                            n_lo=n_lo, n_hi=n_hi,
                            lo_base=lo_base, hi_base=hi_base))
    return batches, c


# --------------------------------------------------------------------------
# host-side preprocessing (sharding)
# --------------------------------------------------------------------------

def _prep(x, edge_index, W1, b1, W2, b2, ccsplit=(36, 42)):
    x = np.asarray(x, F32)
    N, FIN = x.shape
    W1 = np.asarray(W1, F32)
    W2 = np.asarray(W2, F32)
    FMID = W1.shape[1]
    FOUT = W2.shape[1]
    assert N % NCORE == 0
    assert FIN == 128 and FOUT == 128 and FMID % 128 == 0
    pcr = N // NCORE
    nblk = (pcr + 127) // 128
    npc = nblk * 128

    src = np.asarray(edge_index[0], np.int64)
    dst = np.asarray(edge_index[1], np.int64)
    deg = np.bincount(dst, minlength=N).astype(np.float64) + 1.0
    inv = 1.0 / np.sqrt(deg)

    loops = np.arange(N, dtype=np.int64)
    srca = np.concatenate([src, loops])
    dsta = np.concatenate([dst, loops])

    k_arr = dsta // pcr
    l_arr = dsta % pcr
    b_arr = l_arr // 128
    d_arr = (l_arr % 128).astype(F32)

    key = (k_arr * nblk + b_arr).astype(np.int64)
    cnt = np.bincount(key, minlength=NCORE * nblk).reshape(NCORE, nblk)

    perm = np.argsort(-cnt, axis=1, kind="stable")
    invperm = np.empty_like(perm)
    for k in range(NCORE):
        invperm[k, perm[k]] = np.arange(nblk)

    # gather-table positions per layer (wrap layout: partition = pos%128,
    # rank = pos//128)
    sk = srca // pcr
    sl = srca % pcr
    sb = sl // 128
    sd = sl % 128
    slot_of = invperm[sk, sb]
    pos = {
        1: srca,                                  # x wrap table [RX*128]
        2: (sk * nblk + slot_of) * 128 + sd,      # t2 wrap table [8*nblk*128]
    }
    RX = (N + 127) // 128
    RT = max(RX, NCORE * nblk)
    tabrows = {1: RX * 128, 2: NCORE * nblk * 128}

    order = np.argsort(key, kind="stable")
    key_s = key[order]
    grp = np.arange(NCORE * nblk)
    starts = np.searchsorted(key_s, grp)
    ends = np.searchsorted(key_s, grp + 1)

    scheds, Cs, budgets = {}, {}, {}
    for L in (1, 2):
        lo_cnt = np.bincount(key[pos[L] < LO_SPLIT],
                             minlength=NCORE * nblk).reshape(NCORE, nblk)
        hi_cnt = cnt - lo_cnt
        lo_p = np.take_along_axis(lo_cnt, perm, 1)
        hi_p = np.take_along_axis(hi_cnt, perm, 1)
        LO = np.ceil(lo_p.max(0) / 128).astype(int)
        HI = np.ceil(hi_p.max(0) / 128).astype(int)
        budgets[L] = (tuple(LO), tuple(HI))
        scheds[L], Cs[L] = _make_schedule(LO, HI, nblk, GROUP)

    per_core = []
    for k in range(NCORE):
        maps = {}
        invd = np.zeros(nblk * 128, np.float64)
        for s in range(nblk):
            b = perm[k, s]
            lo = b * 128
            hi = min(lo + 128, pcr)
            if hi > lo:
                invd[s * 128:s * 128 + (hi - lo)] = inv[k * pcr + lo:k * pcr + hi]
        maps["invdb"] = np.ascontiguousarray(
            np.broadcast_to(invd.astype(BF16), (128, nblk * 128)))
        for L in (1, 2):
            C = Cs[L]
            idxf = np.zeros(C * 128, np.int32)
            df = np.full(C * 128, -1.0, F32)
            pL = pos[L]
            for batch in scheds[L]:
                for kind, entries in (("lo", batch["lo"]), ("hi", batch["hi"])):
                    for (j, _col, gc, nch) in entries:
                        if nch == 0:
                            continue
                        b = perm[k, j]
                        g = k * nblk + b
                        rows = order[starts[g]:ends[g]]
                        p = pL[rows]
                        sel = rows[p < LO_SPLIT] if kind == "lo" else rows[p >= LO_SPLIT]
                        sel = sel[np.argsort(pL[sel], kind="stable")]
                        m = len(sel)
                        assert m <= nch * 128
                        base = gc * 128
                        pp = pL[sel]
                        if kind == "hi":
                            pp = pp - LO_SPLIT
                        idxf[base:base + m] = pp
                        df[base:base + m] = d_arr[sel]
            assert idxf.max(initial=0) < min(LO_SPLIT, tabrows[L]) and idxf.min(initial=0) >= 0
            idx16 = np.ascontiguousarray(
                np.tile(idxf.astype(np.int16).reshape(-1, 16).T, (NCORE, 1)))
            maps[f"idx{L}"] = idx16
            maps[f"_idxf{L}"] = idxf.copy()
            maps[f"_df{L}"] = df.copy()
            # per-chunk dst offsets, [128 e, C]; pad -> -1 (is_equal never hits)
            maps[f"m{L}d"] = np.ascontiguousarray(
                df.reshape(C, 128).T.astype(F32))
        per_core.append(maps)

    nh = FMID // 128
    xs = (x.astype(np.float64) * inv[:, None]).astype(BF16)
    # wrap layout: xw[p, r*128+f] = xs[r*128+p, f]
    xpad = np.zeros((RX * 128, FIN), BF16)
    xpad[:N] = xs
    xw = np.ascontiguousarray(
        xpad.reshape(RX, 128, FIN).transpose(1, 0, 2).reshape(128, RX * FIN))
    iota = np.ascontiguousarray(
        np.broadcast_to(np.arange(128, dtype=F32), (128, 128)).copy())
    shared = {
        "xw": xw,
        "w1t": np.ascontiguousarray(W1.astype(BF16)),
        "w2t": np.ascontiguousarray(W2.astype(BF16)),
        "b1t": np.ascontiguousarray(np.asarray(b1, F32).reshape(nh, 128).T),
        "b2t": np.ascontiguousarray(np.asarray(b2, F32).reshape(1, 128).T),
        "ident": np.ascontiguousarray(np.eye(128, dtype=BF16)),
        "iota": iota,
    }

    dims = dict(N=N, FIN=FIN, FMID=FMID, FOUT=FOUT,
                pcr=pcr, nblk=nblk, npc=npc, RX=RX, RT=RT)
    return dims, scheds, Cs, budgets, shared, per_core


# --------------------------------------------------------------------------
# bass program
# --------------------------------------------------------------------------

def _build(dims, scheds, Cs, collective=True, ccsplit=(36, 42),
           copy_split=False, debug=False, debug_dump=False):
    N, FIN, FMID, FOUT = dims["N"], dims["FIN"], dims["FMID"], dims["FOUT"]
    nblk, npc, RX, RT = dims["nblk"], dims["npc"], dims["RX"], dims["RT"]
    nh = FMID // 128
    dt = mybir.dt

    nc = bacc.Bacc("TRN2", num_devices=NCORE,
                   dynamic_dma_scratch_size=SWDGE_SCRATCH,
                   num_swdge_queues=NQUEUES, debug=debug)
    qctr = [0]

    xw = nc.declare_dram_parameter("xw", [128, RX * FIN], dt.bfloat16, False)
    w1 = nc.declare_dram_parameter("w1t", [128, FMID], dt.bfloat16, False)
    w2 = nc.declare_dram_parameter("w2t", [FMID, FOUT], dt.bfloat16, False)
    b1 = nc.declare_dram_parameter("b1t", [128, nh], dt.float32, False)
    b2 = nc.declare_dram_parameter("b2t", [128, 1], dt.float32, False)
    ident = nc.declare_dram_parameter("ident", [128, 128], dt.bfloat16, False)
    iota_d = nc.declare_dram_parameter("iota", [128, 128], dt.float32, False)
    invdb = nc.declare_dram_parameter("invdb", [128, nblk * 128], dt.bfloat16, False)
    idx_d, md_d = {}, {}
    for L in (1, 2):
        C = Cs[L]
        idx_d[L] = nc.declare_dram_parameter(f"idx{L}", [128, C * 8], dt.int16, False)
        md_d[L] = nc.declare_dram_parameter(f"m{L}d", [128, C], dt.float32, False)
    pooled_d = nc.declare_dram_parameter("pooled", [128, 1], dt.float32, True)
    agg_dbg = msb_dbg = s_dbg = None
    if debug_dump:
        agg_dbg = nc.declare_dram_parameter("aggdbg", [128, nblk * 128],
                                            dt.bfloat16, True)
        msb_dbg = nc.declare_dram_parameter("msbdbg", [128, Cs[1] * 128],
                                            dt.bfloat16, True)
        s_dbg = nc.declare_dram_parameter("sdbg", [128, Cs[1] * 128],
                                          dt.bfloat16, True)

    bounds = [0] + list(ccsplit) + [nblk]
    regw = [bounds[r + 1] - bounds[r] for r in range(len(bounds) - 1)]
    t2_loc = [nc.dram_tensor(f"t2loc{r}", [128, w, FOUT], dt.bfloat16)
              for r, w in enumerate(regw)]
    t2_full = nc.dram_tensor("t2full", [NCORE * 128 * nblk, FOUT], dt.bfloat16,
                             addr_space="Shared")

    batch_ends = {b["slots"][-1] + 1 for b in scheds[1]}
    assert all(b in batch_ends for b in bounds[1:]), (bounds, sorted(batch_ends))

    with TileContext(nc) as tc, ExitStack() as ctx:
        constp = ctx.enter_context(tc.tile_pool(name="const", bufs=1))
        msgp = ctx.enter_context(tc.tile_pool(name="msg", bufs=3))
        msbp = ctx.enter_context(tc.tile_pool(name="msb", bufs=3))
        sbp = ctx.enter_context(tc.tile_pool(name="sbld", bufs=3))
        workp = ctx.enter_context(tc.tile_pool(name="work", bufs=3))
        metap = ctx.enter_context(tc.tile_pool(name="meta", bufs=2))
        ptg = ctx.enter_context(tc.tile_pool(name="ptg", bufs=2, space="PSUM"))
        psa = ctx.enter_context(tc.tile_pool(name="psa", bufs=2, space="PSUM"))
        psh = ctx.enter_context(tc.tile_pool(name="psh", bufs=1, space="PSUM"))
        pst = ctx.enter_context(tc.tile_pool(name="pst", bufs=1, space="PSUM"))

        def load(tag, shape, dtype, src_ap):
            t = constp.tile(shape, dtype, tag=tag)
            nc.sync.dma_start(out=t[:], in_=src_ap)
            return t

        w1_sb = load("w1c", [128, FMID], dt.bfloat16, w1[:])
        w2_sb = load("w2c", [128, nh, FOUT], dt.bfloat16,
                     w2[:].rearrange("(h k) n -> k h n", h=nh))
        b1_sb = load("b1c", [128, nh], dt.float32, b1[:])
        b2_sb = load("b2c", [128, 1], dt.float32, b2[:])
        ident_sb = load("identc", [128, 128], dt.bfloat16, ident[:])
        iota_sb = load("iotac", [128, 128], dt.float32, iota_d[:])
        MBC = max(b["n_lo"] + b["n_hi"] for L in (1, 2) for b in scheds[L])
        pooled_sb = constp.tile([128, nblk], dt.float32, tag="pooledc")

        # the shared gather table: x wrap first, overwritten by t2 wrap
        tabA = constp.tile([128, RT * 128], dt.bfloat16, tag="tabA")
        nc.sync.dma_start(out=tabA[:, :LO_SPLIT], in_=xw[:, :LO_SPLIT])
        nc.sync.dma_start(out=tabA[:, LO_SPLIT:RX * 128],
                          in_=xw[:, LO_SPLIT:])

        gctr = [0]  # copy-engine alternation counter

        def do_layer(L, tab_lo, tab_hi):
            for batch in scheds[L]:
                c0 = batch["lo_base"]
                nb = batch["n_lo"] + batch["n_hi"]
                s0 = batch["slots"][0]
                nsl = len(batch["slots"])
                idxb = metap.tile([128, MBC * 8], dt.int16, tag="idxb")
                nc.sync.dma_start(out=idxb[:, :nb * 8],
                                  in_=idx_d[L][:, c0 * 8:(c0 + nb) * 8])
                mdb = metap.tile([128, MBC], dt.float32, tag="mdb")
                nc.sync.dma_start(out=mdb[:, :nb],
                                  in_=md_d[L][:, c0:c0 + nb])
                invb = metap.tile([128, GROUP * 128], dt.bfloat16, tag="invb")
                nc.sync.dma_start(out=invb[:, :nsl * 128],
                                  in_=invdb[:, s0 * 128:(s0 + nsl) * 128])
                # gather calls (transposed, SBUF source), emitted lazily in
                # consumption order so the msg tile ring can't deadlock;
                # chunk -> (tile, off)
                cmap = {}
                call_lists = {
                    "lo": [(batch["lo_base"] + a,
                            min(GCAP, batch["n_lo"] - a), tab_lo)
                           for a in range(0, batch["n_lo"], GCAP)],
                    "hi": [(batch["hi_base"] + a,
                            min(GCAP, batch["n_hi"] - a), tab_hi)
                           for a in range(0, batch["n_hi"], GCAP)],
                }
                cptr = {"lo": 0, "hi": 0}

                def ensure_calls(kind, gc_end):
                    cl = call_lists[kind]
                    while cptr[kind] < len(cl) and cl[cptr[kind]][0] < gc_end:
                        cbase, cn, tab = cl[cptr[kind]]
                        mt = msgp.tile([128, 1, GCAP * 128], dt.bfloat16,
                                       tag="mt")
                        nc.gpsimd.dma_gather(
                            mt[:, :, :cn * 128], tab,
                            idxb[:, (cbase - c0) * 8:(cbase + cn - c0) * 8],
                            cn * 128, cn * 128, 128,
                            transpose=True,
                            single_packet=False,
                            queue_num=qctr[0] % NQUEUES,
                            sbuf_tokens_per_rank=128,
                            sbuf_free_dim_per_rank=256,
                            sbuf_free_dim_pad_per_rank=0,
                            sbuf_byte_offset=0)
                        qctr[0] += 1
                        for i in range(cn):
                            cmap[cbase + i] = (mt, i)
                        cptr[kind] += 1

                # transpose / copy / S groups of TG chunks, built lazily
                smap = {}

                def build_group(kind, g0, w):
                    ensure_calls(kind, g0 + w)
                    tp = ptg.tile([128, TG * 128], dt.bfloat16, tag="tp")
                    for i in range(w):
                        mt, off = cmap[g0 + i]
                        nc.tensor.transpose(
                            tp[:, i * 128:(i + 1) * 128],
                            mt[:, 0, off * 128:(off + 1) * 128], ident_sb[:])
                    msb = msbp.tile([128, TG * 128], dt.bfloat16, tag="msb")
                    if copy_split and gctr[0] % 2:
                        nc.vector.tensor_copy(msb[:, :w * 128], tp[:, :w * 128])
                    else:
                        nc.scalar.copy(msb[:, :w * 128], tp[:, :w * 128])
                    gctr[0] += 1
                    s_t = sbp.tile([128, TG * 128], dt.bfloat16, tag="st")
                    for i in range(w):
                        nc.vector.tensor_scalar(
                            out=s_t[:, i * 128:(i + 1) * 128], in0=iota_sb[:],
                            scalar1=mdb[:, g0 + i - c0:g0 + i - c0 + 1],
                            scalar2=None, op0=ALU.is_equal)
                    if L == 1 and msb_dbg is not None:
                        nc.sync.dma_start(
                            out=msb_dbg[:, g0 * 128:(g0 + w) * 128],
                            in_=msb[:, :w * 128])
                        nc.sync.dma_start(
                            out=s_dbg[:, g0 * 128:(g0 + w) * 128],
                            in_=s_t[:, :w * 128])
                    for i in range(w):
                        smap[g0 + i] = (msb, s_t, i)

                groups = {}
                for kind, base, n in (("lo", batch["lo_base"], batch["n_lo"]),
                                      ("hi", batch["hi_base"], batch["n_hi"])):
                    groups[kind] = [(base + g0, min(TG, n - g0))
                                    for g0 in range(0, n, TG)]
                gptr = {"lo": 0, "hi": 0}

                def ensure_groups(kind, gc_end):
                    gl = groups[kind]
                    while gptr[kind] < len(gl) and gl[gptr[kind]][0] < gc_end:
                        build_group(kind, *gl[gptr[kind]])
                        gptr[kind] += 1

                t2b = None
                if L == 1:
                    t2b = workp.tile([128, len(batch["slots"]), FOUT],
                                     dt.bfloat16, tag="t2b")
                for jj, ((j, colL, gcL, nL), (_j2, colH, gcH, nH)) in enumerate(
                        zip(batch["lo"], batch["hi"])):
                    total = nL + nH
                    ensure_groups("lo", gcL + nL)
                    ensure_groups("hi", gcH + nH)
                    agg = psa.tile([128, 128], dt.float32, tag="agg")
                    if total == 0:
                        nc.vector.memset(agg[:], 0.0)
                    ci = 0
                    for gc0, n in ((gcL, nL), (gcH, nH)):
                        for i in range(n):
                            msb, s_t, c = smap[gc0 + i]
                            nc.tensor.matmul(
                                agg[:],
                                lhsT=msb[:, c * 128:(c + 1) * 128],
                                rhs=s_t[:, c * 128:(c + 1) * 128],
                                start=(ci == 0), stop=(ci == total - 1))
                            ci += 1
                    inv_sl = invb[:, jj * 128:(jj + 1) * 128]
                    if L == 1:
                        agg_sb = workp.tile([128, 128], dt.bfloat16,
                                            tag="aggsb")
                        nc.vector.tensor_tensor(out=agg_sb[:], in0=agg[:],
                                                in1=inv_sl, op=ALU.mult)
                        if agg_dbg is not None:
                            nc.sync.dma_start(
                                out=agg_dbg[:, j * 128:(j + 1) * 128],
                                in_=agg_sb[:])
                        h1p = psh.tile([128, nh, 128], dt.float32, tag="h1")
                        for h in range(nh):
                            nc.tensor.matmul(h1p[:, h, :],
                                             lhsT=w1_sb[:, 128 * h:128 * (h + 1)],
                                             rhs=agg_sb[:], start=True, stop=True)
                        h1_sb = workp.tile([128, nh, 128], dt.bfloat16, tag="h1sb")
                        for h in range(nh):
                            nc.scalar.activation(h1_sb[:, h, :], h1p[:, h, :],
                                                 AF.Relu, bias=b1_sb[:, h:h + 1],
                                                 scale=1.0)
                        t2p = pst.tile([128, 128], dt.float32, tag="t2")
                        for h in range(nh):
                            nc.tensor.matmul(t2p[:], lhsT=h1_sb[:, h, :],
                                             rhs=w2_sb[:, h, :],
                                             start=(h == 0), stop=(h == nh - 1))
                        nc.vector.tensor_tensor(out=t2b[:, jj, :], in0=t2p[:],
                                                in1=inv_sl, op=ALU.mult)
                    else:
                        agg2s = workp.tile([128, 128], dt.float32, tag="agg2s")
                        nc.vector.tensor_tensor(out=agg2s[:], in0=agg[:],
                                                in1=inv_sl, op=ALU.mult)
                        scr = workp.tile([128, 128], dt.bfloat16, tag="scr")
                        nc.scalar.activation(scr[:], agg2s[:], AF.Relu,
                                             bias=b2_sb[:, 0:1], scale=1.0,
                                             accum_out=pooled_sb[:, j:j + 1])
                if L == 1:
                    s0 = batch["slots"][0]
                    send = s0 + len(batch["slots"])
                    r = next(i for i in range(len(regw))
                             if bounds[i] <= s0 < bounds[i + 1])
                    assert send <= bounds[r + 1]
                    nc.sync.dma_start(
                        out=t2_loc[r][:, s0 - bounds[r]:send - bounds[r], :],
                        in_=t2b[:])
                if L == 1 and collective:
                    send = batch["slots"][-1] + 1
                    for r in range(len(regw)):
                        if bounds[r + 1] == send:
                            W = regw[r]
                            off = NCORE * 128 * bounds[r]
                            sz = NCORE * 128 * W
                            nc.gpsimd.collective_compute(
                                "AllGather", ALU.bypass,
                                replica_groups=[list(range(NCORE))],
                                ins=[t2_loc[r][:]],
                                outs=[t2_full[off:off + sz, :]])

        do_layer(1, tabA[:, :LO_SPLIT], tabA[:, LO_SPLIT:RX * 128])
        # land each AllGathered region into the wrap table: DRAM rows
        # (k d s) f -> SBUF [d, k, (s f)]. Emitted only after ALL layer-1
        # gathers (WAR on tabA): these overwrite the x table with t2.
        for r in range(len(regw)):
            W = regw[r]
            off = NCORE * 128 * bounds[r]
            sz = NCORE * 128 * W
            src = t2_full[off:off + sz, :].rearrange(
                "(k d s) f -> d k (s f)", k=NCORE, d=128, s=W)
            dst = tabA[:].rearrange(
                "p (k sf) -> p k sf",
                k=NCORE)[:, :, bounds[r] * 128:(bounds[r] + W) * 128]
            nc.sync.dma_start(out=dst, in_=src)
        do_layer(2, tabA[:, :LO_SPLIT], tabA[:, LO_SPLIT:RT * 128])

        pout = workp.tile([128, 1], dt.float32, tag="po")
        nc.vector.tensor_reduce(pout[:], pooled_sb[:],
                                axis=mybir.AxisListType.X, op=ALU.add)
        nc.sync.dma_start(out=pooled_d[:], in_=pout[:])

    nc.compile()
    return nc


# --------------------------------------------------------------------------
# entry point
# --------------------------------------------------------------------------

_CACHE = {}


def _get_program(x, edge_index, W1, b1, W2, b2):
    dims, scheds, Cs, budgets, shared, per_core = _prep(
        x, edge_index, W1, b1, W2, b2)
    key = (dims["N"], dims["FIN"], dims["FMID"], dims["FOUT"],
           budgets[1], budgets[2])
    if key not in _CACHE:
        _CACHE[key] = _build(dims, scheds, Cs)
    return _CACHE[key], dims, shared, per_core


def kernel(x, edge_index, W1, b1, W2, b2, Wfc, bfc):
    global LAST_EXEC_NS, LAST_RESULTS
    nc, dims, shared, per_core = _get_program(x, edge_index, W1, b1, W2, b2)

    in_maps = []
    for k in range(NCORE):
        m = dict(shared)
        m.update(per_core[k])
        in_maps.append(m)

    kw = {}
    if TRACE:
        kw["trace"] = True
    res = run_bass_kernel_spmd(nc, in_maps, core_ids=list(range(NCORE)), **kw)
    LAST_RESULTS = res
    LAST_EXEC_NS = getattr(res, "exec_time_ns", None)

    partials = np.stack([np.asarray(res.results[k]["pooled"], np.float64)[:, 0]
                         for k in range(NCORE)])
    pooled_sum = partials.sum(0)

    n_pads = NCORE * (dims["npc"] - dims["pcr"])
    relu_b2 = np.maximum(np.asarray(b2, np.float64), 0.0)
    pooled = (pooled_sum - n_pads * relu_b2) / dims["N"]
    out = pooled @ np.asarray(Wfc, np.float64) + np.asarray(bfc, np.float64)
    return out.astype(F32)
